# revision 29
# baseline (speedup 1.0000x reference)
"""Trainium2 Bass kernel for the DLI (dialogue-turn ordering) loss.

Math (exact reduction of the reference):
  With 2 classes, NLL(label y) = softplus(l_{1-y} - l_y).
  u[b,j] = enc[b,j] @ (W[:D,1]-W[:D,0]),
  v[b,k] = enc[b,k] @ (W[D:,1]-W[D:,0]),
  c      = b[1]-b[0],  d[b,j,k] = u[b,j] + v[b,k] + c
  label = 1 iff k == j-1; valid pairs: k < j < len_b;  softplus(-d) = softplus(d) - d
  =>  sum_nll = sum_{valid} softplus(d) - sum_{b, 1<=j<len_b} d[b,j,j-1]
  loss = sum_nll / max(n_valid, 1)

Sharding: data-parallel over batch (64 -> 8 cores x 8); per core the 8
batches form G=2 groups of 4 (group width 512 = one PSUM bank, the
matmul-N ISA cap).

Device strategy (PE-centric, fp8 DoubleRow, fused v-broadcast):
  * Host pre-packs enc as fp8e4m3 in DoubleRow layout [p, (g, c, t, b, j)]
    with d = c*256 + t*128 + p (rel err 5.6e-5, 350x under tolerance);
    HBM traffic is 2 MB/core, streamed as 8 x 256KB chunks alternating
    across both HWDGE queues at ~340 GB/s.
  * The DoubleRow lhsT [128, 2, 128] has col 0 = 64*wu, cols 1..127 =
    64*wv: one stream computes u (psum row 0) AND broadcasts v into rows
    1..127 of psum_d for free (row 0's pairs are all tri-masked anyway).
  * psum_d[j,(b,k)] accumulation: tri-mask matmul (ident x trineg_rep,
    trineg row 32 zeroed) -> 8 DoubleRow matmuls -> 4 rank-1
    ju-broadcasts. ju = psum row 0 + aux (DVE STT over psum rows 0..32
    also extracts v*64 from row 32 in the same pass; aux compensates the
    row-0 tri-NEG and carries the length-mask + c, all in the x64
    domain; Exp descales via scale=1/64).
  * Row 32 stays unmasked on device; it is excluded from the on-device
    reduction (zeroed ones-column) and the host adds its true
    contribution from the ju/v rows it receives anyway.
  * ACT: per-group Exp then Ln(x+1) with accum_out (the Softplus table
    on this build is mislabeled garbage - measured 2x+31). One PE
    matmul reduces the accum column to [1, G] so the final DMA is one
    fat descriptor (128 x 8B descriptors cost ~4us of write-receipt
    before teardown can start).
  * Output DMAs ride the sync queue: DMA_DIRECT2D issue occupies the
    issuing engine's sequencer ~0.6us, which stalled ACT when they were
    on the scalar queue.
  * PE warms up on memset junk before any DMA lands (p-state ramp:
    0.65 -> 1.2 -> 2.4 GHz with ~3us of continuous busy).
  * GpSimd is untouched: SWDGE was measured strictly worse (7.5us Q7
    drain + later const arrival).
  * Host: loss = (rs - diag(ju, v) + row32) / n_valid in f64.

Span budget per core (~30-32us total): ~6.5us NEFF preamble, ~8.5us DMA
stream, ~2us last-chunk receipt, ~4.5us compute tail, ~1.5us rs
receipt, ~7.5us teardown (a hardcoded full-sem-file wipe split across
engines plus final barriers - not controllable from bass).
"""

import glob
import json
import os
import shutil
import sys
import tempfile

if "/opt/trn_rl_repo" not in sys.path:
    sys.path.insert(0, "/opt/trn_rl_repo")

_ACT_TABLE = "natural_log_exp_and_others"


def _force_combined_act_table():
    """Point walrus at an act_info.json holding only natural_log_exp_and_others
    (contains exp+ln+copy), so every ACTIVATE shares one table."""
    if os.environ.get("BASS_ACT_ROOT_JSON_PATH"):
        return
    from neuronxcc.driver.Job import Job  # type: ignore

    pwp = None
    for cand in glob.glob(os.path.join(Job.getPackageDir(), "pwp", "pwp_bin_*")):
        if os.path.exists(os.path.join(cand, "act_info.json")):
            pwp = cand
            break
    if pwp is None:
        return
    info = json.load(open(os.path.join(pwp, "act_info.json")))
    keep = [t for t in info.get("act_func_sets", []) if t.get("name") == _ACT_TABLE]
    if not keep:
        return
    out_dir = os.path.join(tempfile.gettempdir(), "dli_act_combined")
    os.makedirs(out_dir, exist_ok=True)
    for t in keep:
        for k in info.get("pwp_file_keys", []):
            f = t.get(k)
            src = os.path.join(pwp, f) if f else None
            if src and os.path.exists(src):
                dst = os.path.join(out_dir, f)
                if not os.path.exists(dst):
                    shutil.copy(src, dst)
    info = dict(info)
    info["act_func_sets"] = keep
    with open(os.path.join(out_dir, "act_info.json"), "w") as f:
        json.dump(info, f)
    os.environ["BASS_ACT_ROOT_JSON_PATH"] = os.path.join(out_dir, "act_info.json")


_force_combined_act_table()

from contextlib import ExitStack

import ml_dtypes
import numpy as np

import concourse.bacc as bacc
import concourse.bass as bass
import concourse.hw_specs as hw_specs
import concourse.mybir as mybir
import concourse.tile as tile

_orig_get_act_tables = hw_specs.get_activation_tables


def _combined_act_tables(module_arch):
    tabs = _orig_get_act_tables(module_arch)
    kept = {k: v for k, v in tabs.items() if k == _ACT_TABLE}
    return kept if kept and os.environ.get("BASS_ACT_ROOT_JSON_PATH") else tabs


hw_specs.get_activation_tables = _combined_act_tables
bacc.get_activation_tables = _combined_act_tables

# Shrink the kernel semaphore range: finalize() emits dma_reset+sem_clear
# over the WHOLE range, which codegen expands into a ~250-instruction
# per-sem clear storm (~7us of teardown). We use ~12 tile sems; a tight
# range keeps the storm proportional.
bass.get_kernel_semaphore_range = lambda: range(150, 172)

# Cheaper kernel teardown (same rationale as previous version).
from concourse.vector_clock import ScopedClock as _ScopedClock


def _cheap_drain_and_barrier(self, tick_clock, wait_clock):
    drain_inst = self.nc.sync.drain()
    wait_clock.add_sem_waits(
        drain_inst.ins, _ScopedClock({None: tick_clock.global_clock})
    )
    self.nc.all_engine_barrier()
    popped = self.nc._tile_sem_poison_stack.pop()
    assert popped is self._sem_poison
    self.nc.clear_and_free_semaphores(list(self.sems.allocated().values()))


tile.TileContext._drain_and_barrier = _cheap_drain_and_barrier

F32 = mybir.dt.float32
BF16 = mybir.dt.bfloat16
FP8 = mybir.dt.float8e4
ALU = mybir.AluOpType
ACTF = mybir.ActivationFunctionType
DR = mybir.MatmulPerfMode.DoubleRow

BSZ, L, D = 64, 128, 2048
N_CORES = 8
NB = BSZ // N_CORES  # batches per core = 8
GROUPS = [4, 4]  # batches per group (matmul N caps at 512 = one psum bank)
G = len(GROUPS)
GOFF = [sum(GROUPS[:i]) for i in range(G + 1)]  # batch offsets
NWS = [bg * L for bg in GROUPS]  # free width per group
NC_CHUNK = 8  # DoubleRow k-chunks (256 d each)
ENC_DMAS = [8, 8]  # single-chunk enc DMAs: finer PE pacing and only one
# 128KB chunk's matmul trails the final DMA receipt
NEG = -30000.0  # additive mask; exp(NEG) == 0 in f32
NEGB = float(ml_dtypes.bfloat16(NEG))  # the value actually landing in psum
VROW = 32  # psum row v is read from (legal engine start partition)
WSCALE = 64.0  # fp8 weight pre-scale (host) / descale (DVE)
N_WARM = 4  # PE p-state warmup matmuls (fill the idle window before cst)


def build_program():
    nc = bacc.Bacc("TRN2", target_bir_lowering=False, debug=False, num_devices=1)

    enc = nc.dram_tensor("enc", [128, NC_CHUNK * 2 * NB * L], FP8,
                         kind="ExternalInput").ap()
    w2 = nc.dram_tensor("w2", [128, NC_CHUNK * 2 * 128], FP8,
                        kind="ExternalInput").ap()
    # cst u16 cols: [0:L] ident bf16 | [L:] trineg bf16 replicated
    # max(GROUPS) times (row VROW zeroed); sliced per group
    cst = nc.dram_tensor("cst", [L, L + max(GROUPS) * L], mybir.dt.uint16,
                         kind="ExternalInput").ap()
    auxju = nc.dram_tensor("auxju", [1, NB * L], F32, kind="ExternalInput").ap()
    juv = nc.dram_tensor("juv", [2, NB * L], BF16, kind="ExternalOutput").ap()
    rs = nc.dram_tensor("rs", [1, G], F32, kind="ExternalOutput").ap()

    with tile.TileContext(nc) as tc, ExitStack() as ctx:
        consts = ctx.enter_context(tc.tile_pool(name="consts", bufs=1))
        rows = ctx.enter_context(tc.tile_pool(name="rows", bufs=2))
        exg_pool = ctx.enter_context(tc.tile_pool(name="exg", bufs=2))
        accs = ctx.enter_context(tc.tile_pool(name="accs", bufs=1))
        ps_d_pool = ctx.enter_context(tc.tile_pool(name="psd", bufs=1, space="PSUM"))
        ps_w_pool = ctx.enter_context(tc.tile_pool(name="psw", bufs=1, space="PSUM"))

        # ---- consts first on the HWDGE queues (SWDGE tried: a 7.5us Q7
        # drain + ~3us slower start make it strictly worse) ----
        cst_sb = consts.tile([L, L + max(GROUPS) * L], mybir.dt.uint16)
        nc.sync.dma_start(cst_sb[:], cst[:])
        ident = cst_sb[:, 0:L].bitcast(BF16)
        trineg_all = cst_sb[:, L:].bitcast(BF16)
        def trineg_rep(bg):
            return trineg_all[:, 0 : bg * L]
        w2_sb = consts.tile([128, NC_CHUNK, 2, 128], FP8)
        nc.scalar.dma_start(w2_sb[:], w2[:])
        # aux33 rows: 0 = aux_ju (f32), 32 = 0 (v passthrough); rows 1-31
        # only feed garbage lanes of the fused extract, memset for safety
        aux33 = consts.tile([VROW + 1, NB * L], F32)
        nc.vector.memset(aux33[:], 0.0)
        nc.scalar.dma_start(aux33[0:1, :], auxju[:], single_packet=True)

        ones_row = consts.tile([1, L], BF16)
        nc.vector.memset(ones_row[:], 1.0)

        # ---- enc: chained chunks alternating across both HWDGE queues ----
        enc_sb = [consts.tile([128, NC_CHUNK, 2, NWS[g]], FP8, name=f"enc{g}")
                  for g in range(G)]
        qi = 0
        for g in range(G):
            dpg = ENC_DMAS[g]
            cpd = NC_CHUNK // dpg  # k-chunks per DMA
            per = cpd * 2 * NWS[g]  # fp8 elems per DMA per partition
            base = GOFF[g] * L * NC_CHUNK * 2
            for h in range(dpg):
                eng = nc.sync if qi % 2 == 0 else nc.scalar
                qi += 1
                eng.dma_start(
                    enc_sb[g][:, h * cpd : (h + 1) * cpd, :, :],
                    enc[:, base + h * per : base + (h + 1) * per],
                )

        O = accs.tile([L, G], F32)
        ones_col = consts.tile([L, 1], F32)
        nc.vector.memset(ones_col[:], 1.0)
        nc.vector.memset(ones_col[VROW : VROW + 1, :], 0.0)

        # ---- PE warmup (memset-fed: starts before any DMA lands) ----
        wjunk = consts.tile([L, 512], BF16)
        nc.vector.memset(wjunk[:], 0.0)
        ps_warm = ps_w_pool.tile([L, 512], F32)
        for _ in range(N_WARM):
            nc.tensor.matmul(ps_warm[:], lhsT=wjunk[:, 0:L], rhs=wjunk[:],
                             start=True, stop=True)
        warm_act = rows.tile([1, 2], BF16, tag="wact")
        nc.scalar.activation(warm_act[:], ones_row[0:1, 0:2], ACTF.Exp)

        ps_d = [ps_d_pool.tile([L, NWS[g]], F32, name=f"psd{g}") for g in range(G)]
        exg = exg_pool.tile([L, NB * L], F32)

        # tri masks early (also warms PE); start=True opens the accumulation
        for g in range(G):
            nc.tensor.matmul(ps_d[g][:], lhsT=ident, rhs=trineg_rep(GROUPS[g]),
                             start=True, stop=False)

        # uv dots: 8 DoubleRow matmuls per group, straight into psum_d.
        # Row 0 accumulates u*64; rows 1..127 accumulate the v*64 broadcast.
        for g in range(G):
            for c in range(NC_CHUNK):
                nc.tensor.matmul(
                    ps_d[g][:],
                    lhsT=w2_sb[:, c, :, :],
                    rhs=enc_sb[g][:, c, :, :],
                    perf_mode=DR,
                    start=False,
                    stop=False,
                    skip_group_check=True,
                )

        for g in range(G):
            NW = NWS[g]
            OFFL = GOFF[g] * L
            # fused extract: one DVE pass over psum rows 0..VROW pulls both
            # ju_s (row 0: tri-NEG + u*64, aux compensates) and v*64 (row
            # VROW: tri there is zero, ju not yet added). Rows 1..31 are
            # garbage and never read. Everything stays in the x64 domain;
            # the Exp descales via scale=1/64 and the host divides by 64.
            vju = rows.tile([VROW + 1, NW], BF16, tag="vju", name=f"vju{g}")
            nc.vector.scalar_tensor_tensor(
                out=vju[:], in0=ps_d[g][0 : VROW + 1, :], scalar=1.0,
                op0=ALU.mult, in1=aux33[:, OFFL : OFFL + NW],
                op1=ALU.add,
            )
            ju_sb = vju
            # ship ju/v back early (sync queue: its sequencer is idle here;
            # scalar-queue DMAs would block ACT's Exp/Ln pipeline)
            nc.sync.dma_start(juv[0:1, OFFL : OFFL + NW], vju[0:1, :])
            nc.sync.dma_start(juv[1:2, OFFL : OFFL + NW],
                              vju[VROW : VROW + 1, :])
            # ju broadcast per batch: psum_d[j, b-block] += ju_b[j]
            # (row VROW keeps its unmasked garbage; it is excluded from the
            # on-device reduction and the host adds its true contribution)
            for b in range(GROUPS[g]):
                nc.tensor.matmul(
                    ps_d[g][:, b * L : (b + 1) * L],
                    lhsT=ju_sb[0:1, b * L : (b + 1) * L],
                    rhs=ones_row[:],
                    start=False, stop=True,
                    skip_group_check=True,
                )
            nc.scalar.activation(exg[:, OFFL : OFFL + NW], ps_d[g][:],
                                 ACTF.Exp, scale=1.0 / WSCALE)
            junk = exg_pool.tile([L, NW], BF16, tag="junk", name=f"junk{g}")
            nc.scalar.activation(junk[:], exg[:, OFFL : OFFL + NW],
                                 ACTF.Ln, bias=1.0,
                                 accum_out=O[:, g : g + 1])

        # reduce O over partitions on PE -> [1, G] so the final DMA is one
        # fat descriptor instead of 128 x 8B (receipt latency gates teardown)
        ps_rs = ps_w_pool.tile([1, G], F32, tag="psrs")
        nc.tensor.matmul(ps_rs[:], lhsT=ones_col[:], rhs=O[:],
                         start=True, stop=True)
        rs_sb = rows.tile([1, G], F32, tag="rssb")
        nc.vector.tensor_copy(rs_sb[:], ps_rs[:])
        # NOTE: issuing this DMA after the tile teardown overlaps its
        # ~1.5us write-receipt with the sem-file wipe, but the completion
        # increment then races the wipe zeroing its own semaphore and can
        # leave the device wedged (observed NRT_EXEC_UNIT_UNRECOVERABLE on
        # a later execution). Keep it tracked inside the tile context.
        # Scalar queue: ACT is finished by now and the Sync sequencer is
        # still busy issuing the juv DMAs.
        nc.scalar.dma_start(rs[:], rs_sb[:], single_packet=True)

    nc.compile()
    return nc


_NC = None


def _get_nc():
    global _NC
    if _NC is None:
        _NC = build_program()
    return _NC


def _prep(encoder_output, mask, W, b):
    """Host-side prep: shard + pack the DoubleRow fp8 layout."""
    W = np.asarray(W, dtype=np.float32)
    b = np.asarray(b, dtype=np.float32).reshape(2)
    mask = np.asarray(mask)
    c = float(b[1] - b[0])
    lens = mask.astype(np.int64).sum(axis=1)  # [BSZ]
    j = np.arange(L)

    wu = (W[:D, 1] - W[:D, 0]) * WSCALE
    wv = (W[D:, 1] - W[D:, 0]) * WSCALE
    # w2[p, c, t, m]: col 0 = wu, cols 1.. = wv; d = c*256 + t*128 + p
    w2 = np.empty((128, NC_CHUNK, 2, 128), dtype=np.float32)
    w2[:] = wv.reshape(NC_CHUNK, 2, 128).transpose(2, 0, 1)[:, :, :, None]
    w2[:, :, :, 0] = wu.reshape(NC_CHUNK, 2, 128).transpose(2, 0, 1)
    w2 = w2.reshape(128, NC_CHUNK * 2 * 128).astype(ml_dtypes.float8_e4m3)

    ident_b = np.eye(L, dtype=ml_dtypes.bfloat16)
    trineg_f = np.where(j[None, :] < j[:, None], 0.0, NEG).astype(np.float32)
    trineg_f[VROW] = 0.0
    trineg_b = np.tile(trineg_f.astype(ml_dtypes.bfloat16), (1, max(GROUPS)))
    cst = np.concatenate([ident_b.view(np.uint16), trineg_b.view(np.uint16)], axis=1)

    enc_f = np.asarray(encoder_output, dtype=np.float32)
    maps = []
    for cid in range(N_CORES):
        sl = slice(cid * NB, (cid + 1) * NB)
        ec = enc_f[sl]  # [8, 128, 2048]
        # per group: [b, j, c, t, p] -> [p, c, t, b, j], concatenated g-major
        parts = []
        for g in range(len(GROUPS)):
            eg = ec[GOFF[g] : GOFF[g + 1]]  # [Bg, 128, 2048]
            pg = eg.reshape(GROUPS[g], L, NC_CHUNK, 2, 128).transpose(4, 2, 3, 0, 1)
            parts.append(pg.reshape(128, NC_CHUNK * 2 * GROUPS[g] * L))
        ep = np.ascontiguousarray(np.concatenate(parts, axis=1)).astype(
            ml_dtypes.float8_e4m3
        )
        lc = lens[sl]  # [NB]
        # aux_ju = 64*(len-mask + c) - NEGB (compensates tri row 0 in psum)
        auxv = (
            (np.where(j[None, :] < lc[:, None], 0.0, NEG) + c) * WSCALE - NEGB
        ).astype(np.float32).reshape(1, NB * L)
        maps.append(
            {
                "enc": ep,
                "w2": w2,
                "cst": np.ascontiguousarray(cst),
                "auxju": np.ascontiguousarray(auxv),
            }
        )
    return maps, lens


def kernel(encoder_output, mask, W, b, _run_kwargs=None):
    from concourse.bass_utils import run_bass_kernel_spmd

    nc = _get_nc()
    maps, lens = _prep(np.asarray(encoder_output), mask, W, b)
    res = run_bass_kernel_spmd(nc, maps, core_ids=list(range(N_CORES)),
                               **(_run_kwargs or {}))
    pair_sum = np.float64(0.0)
    diag = np.float64(0.0)
    for cid, r in enumerate(res.results):
        pair_sum += np.asarray(r["rs"], dtype=np.float64).sum()
        juv = np.asarray(r["juv"], dtype=np.float64)  # [2, NB*L]
        for bi in range(NB):
            ln = int(lens[cid * NB + bi])
            ju = juv[0, bi * L : bi * L + L] / WSCALE
            v = juv[1, bi * L : bi * L + L] / WSCALE
            diag += ju[1:ln].sum() + v[: ln - 1].sum()
            # row VROW is excluded from the device reduction (left unmasked);
            # add its true pair contribution here
            if ln > VROW:
                pair_sum += np.log1p(np.exp(ju[VROW] + v[:VROW])).sum()
    n_valid = int((lens * (lens - 1) // 2).sum())
    loss = (pair_sum - diag) / max(n_valid, 1)
    out = np.array(loss, dtype=np.float32)
    if _run_kwargs is not None:
        return out, res
    return out


if __name__ == "__main__":
    import reference

    inputs = {k: np.asarray(v) for k, v in reference.setup_inputs().items()}
    print(kernel(**inputs))


# revision 30
# speedup vs baseline: 1.0364x; 1.0364x over previous
"""Trainium2 Bass kernel for the DLI (dialogue-turn ordering) loss.

Math (exact reduction of the reference):
  With 2 classes, NLL(label y) = softplus(l_{1-y} - l_y).
  u[b,j] = enc[b,j] @ (W[:D,1]-W[:D,0]),
  v[b,k] = enc[b,k] @ (W[D:,1]-W[D:,0]),
  c      = b[1]-b[0],  d[b,j,k] = u[b,j] + v[b,k] + c
  label = 1 iff k == j-1; valid pairs: k < j < len_b;  softplus(-d) = softplus(d) - d
  =>  sum_nll = sum_{valid} softplus(d) - sum_{b, 1<=j<len_b} d[b,j,j-1]
  loss = sum_nll / max(n_valid, 1)

Sharding: data-parallel over batch (64 -> 8 cores x 8); per core the 8
batches form G=2 groups of 4 (group width 512 = one PSUM bank, the
matmul-N ISA cap).

Device strategy (PE-centric, fp8 DoubleRow, fused v-broadcast):
  * Host pre-packs enc as fp8e4m3 in DoubleRow layout [p, (g, c, t, b, j)]
    with d = c*256 + t*128 + p (rel err 5.6e-5, 350x under tolerance);
    HBM traffic is 2 MB/core, streamed as 8 x 256KB chunks alternating
    across both HWDGE queues at ~340 GB/s.
  * The DoubleRow lhsT [128, 2, 128] has col 0 = 64*wu, cols 1..127 =
    64*wv: one stream computes u (psum row 0) AND broadcasts v into rows
    1..127 of psum_d for free (row 0's pairs are all tri-masked anyway).
  * psum_d[j,(b,k)] accumulation: tri-mask matmul (ident x trineg_rep,
    trineg row 32 zeroed) -> 8 DoubleRow matmuls -> 4 rank-1
    ju-broadcasts. ju = psum row 0 + aux (DVE STT over psum rows 0..32
    also extracts v*64 from row 32 in the same pass; aux compensates the
    row-0 tri-NEG and carries the length-mask + c, all in the x64
    domain; Exp descales via scale=1/64).
  * Row 32 stays unmasked on device; it is excluded from the on-device
    reduction (zeroed ones-column) and the host adds its true
    contribution from the ju/v rows it receives anyway.
  * ACT: per-group Exp then Ln(x+1) with accum_out (the Softplus table
    on this build is mislabeled garbage - measured 2x+31). One PE
    matmul reduces the accum column to [1, G] so the final DMA is one
    fat descriptor (128 x 8B descriptors cost ~4us of write-receipt
    before teardown can start).
  * Output DMAs ride the sync queue: DMA_DIRECT2D issue occupies the
    issuing engine's sequencer ~0.6us, which stalled ACT when they were
    on the scalar queue.
  * PE warms up on memset junk before any DMA lands (p-state ramp:
    0.65 -> 1.2 -> 2.4 GHz with ~3us of continuous busy).
  * GpSimd is untouched: SWDGE was measured strictly worse (7.5us Q7
    drain + later const arrival).
  * Host: loss = (rs - diag(ju, v) + row32) / n_valid in f64.

Span budget per core (~30-32us total): ~6.5us NEFF preamble, ~8.5us DMA
stream, ~2us last-chunk receipt, ~4.5us compute tail, ~1.5us rs
receipt, ~7.5us teardown (a hardcoded full-sem-file wipe split across
engines plus final barriers - not controllable from bass).
"""

import glob
import json
import os
import shutil
import sys
import tempfile

if "/opt/trn_rl_repo" not in sys.path:
    sys.path.insert(0, "/opt/trn_rl_repo")

_ACT_TABLE = "natural_log_exp_and_others"


def _force_combined_act_table():
    """Point walrus at an act_info.json holding only natural_log_exp_and_others
    (contains exp+ln+copy), so every ACTIVATE shares one table."""
    if os.environ.get("BASS_ACT_ROOT_JSON_PATH"):
        return
    from neuronxcc.driver.Job import Job  # type: ignore

    pwp = None
    for cand in glob.glob(os.path.join(Job.getPackageDir(), "pwp", "pwp_bin_*")):
        if os.path.exists(os.path.join(cand, "act_info.json")):
            pwp = cand
            break
    if pwp is None:
        return
    info = json.load(open(os.path.join(pwp, "act_info.json")))
    keep = [t for t in info.get("act_func_sets", []) if t.get("name") == _ACT_TABLE]
    if not keep:
        return
    out_dir = os.path.join(tempfile.gettempdir(), "dli_act_combined")
    os.makedirs(out_dir, exist_ok=True)
    for t in keep:
        for k in info.get("pwp_file_keys", []):
            f = t.get(k)
            src = os.path.join(pwp, f) if f else None
            if src and os.path.exists(src):
                dst = os.path.join(out_dir, f)
                if not os.path.exists(dst):
                    shutil.copy(src, dst)
    info = dict(info)
    info["act_func_sets"] = keep
    with open(os.path.join(out_dir, "act_info.json"), "w") as f:
        json.dump(info, f)
    os.environ["BASS_ACT_ROOT_JSON_PATH"] = os.path.join(out_dir, "act_info.json")


_force_combined_act_table()

from contextlib import ExitStack

import ml_dtypes
import numpy as np

import concourse.bacc as bacc
import concourse.bass as bass
import concourse.hw_specs as hw_specs
import concourse.mybir as mybir
import concourse.tile as tile

_orig_get_act_tables = hw_specs.get_activation_tables


def _combined_act_tables(module_arch):
    tabs = _orig_get_act_tables(module_arch)
    kept = {k: v for k, v in tabs.items() if k == _ACT_TABLE}
    return kept if kept and os.environ.get("BASS_ACT_ROOT_JSON_PATH") else tabs


hw_specs.get_activation_tables = _combined_act_tables
bacc.get_activation_tables = _combined_act_tables

# Shrink the kernel semaphore range: finalize() emits dma_reset+sem_clear
# over the WHOLE range, which codegen expands into a ~250-instruction
# per-sem clear storm (~7us of teardown). We use ~12 tile sems; a tight
# range keeps the storm proportional.
bass.get_kernel_semaphore_range = lambda: range(150, 172)

# Cheaper kernel teardown (same rationale as previous version).
from concourse.vector_clock import ScopedClock as _ScopedClock


def _cheap_drain_and_barrier(self, tick_clock, wait_clock):
    drain_inst = self.nc.sync.drain()
    wait_clock.add_sem_waits(
        drain_inst.ins, _ScopedClock({None: tick_clock.global_clock})
    )
    self.nc.all_engine_barrier()
    popped = self.nc._tile_sem_poison_stack.pop()
    assert popped is self._sem_poison
    self.nc.clear_and_free_semaphores(list(self.sems.allocated().values()))


tile.TileContext._drain_and_barrier = _cheap_drain_and_barrier

F32 = mybir.dt.float32
BF16 = mybir.dt.bfloat16
FP8 = mybir.dt.float8e4
ALU = mybir.AluOpType
ACTF = mybir.ActivationFunctionType
DR = mybir.MatmulPerfMode.DoubleRow

BSZ, L, D = 64, 128, 2048
N_CORES = 8
NB = BSZ // N_CORES  # batches per core = 8
GROUPS = [4, 4]  # batches per group (matmul N caps at 512 = one psum bank)
G = len(GROUPS)
GOFF = [sum(GROUPS[:i]) for i in range(G + 1)]  # batch offsets
NWS = [bg * L for bg in GROUPS]  # free width per group
NC_CHUNK = 8  # DoubleRow k-chunks (256 d each)
ENC_DMAS = [4, 8]  # enc DMAs per group; g1 single-chunk so only one
# 128KB chunk's matmul trails the final DMA receipt ([8,8] measured worse:
# extra issue overhead/receipts on the g0 side)
NEG = -30000.0  # additive mask; exp(NEG) == 0 in f32
NEGB = float(ml_dtypes.bfloat16(NEG))  # the value actually landing in psum
VROW = 32  # psum row v is read from (legal engine start partition)
WSCALE = 64.0  # fp8 weight pre-scale (host) / descale (DVE)
N_WARM = 4  # PE p-state warmup matmuls (fill the idle window before cst)


def build_program():
    nc = bacc.Bacc("TRN2", target_bir_lowering=False, debug=False, num_devices=1)

    enc = nc.dram_tensor("enc", [128, NC_CHUNK * 2 * NB * L], FP8,
                         kind="ExternalInput").ap()
    w2 = nc.dram_tensor("w2", [128, NC_CHUNK * 2 * 128], FP8,
                        kind="ExternalInput").ap()
    # cst u16 cols: [0:L] ident bf16 | [L:] trineg bf16 replicated
    # max(GROUPS) times (row VROW zeroed); sliced per group
    cst = nc.dram_tensor("cst", [L, L + max(GROUPS) * L], mybir.dt.uint16,
                         kind="ExternalInput").ap()
    auxju = nc.dram_tensor("auxju", [1, NB * L], F32, kind="ExternalInput").ap()
    juv = nc.dram_tensor("juv", [2, NB * L], BF16, kind="ExternalOutput").ap()
    rs = nc.dram_tensor("rs", [1, G], F32, kind="ExternalOutput").ap()

    with tile.TileContext(nc) as tc, ExitStack() as ctx:
        consts = ctx.enter_context(tc.tile_pool(name="consts", bufs=1))
        rows = ctx.enter_context(tc.tile_pool(name="rows", bufs=2))
        exg_pool = ctx.enter_context(tc.tile_pool(name="exg", bufs=2))
        accs = ctx.enter_context(tc.tile_pool(name="accs", bufs=1))
        ps_d_pool = ctx.enter_context(tc.tile_pool(name="psd", bufs=1, space="PSUM"))
        ps_w_pool = ctx.enter_context(tc.tile_pool(name="psw", bufs=1, space="PSUM"))

        # ---- consts first on the HWDGE queues (SWDGE tried: a 7.5us Q7
        # drain + ~3us slower start make it strictly worse) ----
        cst_sb = consts.tile([L, L + max(GROUPS) * L], mybir.dt.uint16)
        nc.sync.dma_start(cst_sb[:], cst[:])
        ident = cst_sb[:, 0:L].bitcast(BF16)
        trineg_all = cst_sb[:, L:].bitcast(BF16)
        def trineg_rep(bg):
            return trineg_all[:, 0 : bg * L]
        w2_sb = consts.tile([128, NC_CHUNK, 2, 128], FP8)
        nc.scalar.dma_start(w2_sb[:], w2[:])
        # aux33 rows: 0 = aux_ju (f32), 32 = 0 (v passthrough); rows 1-31
        # only feed garbage lanes of the fused extract, memset for safety
        aux33 = consts.tile([VROW + 1, NB * L], F32)
        nc.vector.memset(aux33[:], 0.0)
        nc.scalar.dma_start(aux33[0:1, :], auxju[:], single_packet=True)

        ones_row = consts.tile([1, L], BF16)
        nc.vector.memset(ones_row[:], 1.0)

        # ---- enc: chained chunks alternating across both HWDGE queues ----
        enc_sb = [consts.tile([128, NC_CHUNK, 2, NWS[g]], FP8, name=f"enc{g}")
                  for g in range(G)]
        qi = 0
        for g in range(G):
            dpg = ENC_DMAS[g]
            cpd = NC_CHUNK // dpg  # k-chunks per DMA
            per = cpd * 2 * NWS[g]  # fp8 elems per DMA per partition
            base = GOFF[g] * L * NC_CHUNK * 2
            for h in range(dpg):
                eng = nc.sync if qi % 2 == 0 else nc.scalar
                qi += 1
                eng.dma_start(
                    enc_sb[g][:, h * cpd : (h + 1) * cpd, :, :],
                    enc[:, base + h * per : base + (h + 1) * per],
                )

        O = accs.tile([L, G], F32)
        ones_col = consts.tile([L, 1], F32)
        nc.vector.memset(ones_col[:], 1.0)
        nc.vector.memset(ones_col[VROW : VROW + 1, :], 0.0)

        # ---- PE warmup (memset-fed: starts before any DMA lands) ----
        wjunk = consts.tile([L, 512], BF16)
        nc.vector.memset(wjunk[:], 0.0)
        ps_warm = ps_w_pool.tile([L, 512], F32)
        for _ in range(N_WARM):
            nc.tensor.matmul(ps_warm[:], lhsT=wjunk[:, 0:L], rhs=wjunk[:],
                             start=True, stop=True)
        warm_act = rows.tile([1, 2], BF16, tag="wact")
        nc.scalar.activation(warm_act[:], ones_row[0:1, 0:2], ACTF.Exp)

        ps_d = [ps_d_pool.tile([L, NWS[g]], F32, name=f"psd{g}") for g in range(G)]
        exg = exg_pool.tile([L, NB * L], F32)

        # tri masks early (also warms PE); start=True opens the accumulation
        for g in range(G):
            nc.tensor.matmul(ps_d[g][:], lhsT=ident, rhs=trineg_rep(GROUPS[g]),
                             start=True, stop=False)

        # uv dots: 8 DoubleRow matmuls per group, straight into psum_d.
        # Row 0 accumulates u*64; rows 1..127 accumulate the v*64 broadcast.
        for g in range(G):
            for c in range(NC_CHUNK):
                nc.tensor.matmul(
                    ps_d[g][:],
                    lhsT=w2_sb[:, c, :, :],
                    rhs=enc_sb[g][:, c, :, :],
                    perf_mode=DR,
                    start=False,
                    stop=False,
                    skip_group_check=True,
                )

        for g in range(G):
            NW = NWS[g]
            OFFL = GOFF[g] * L
            # fused extract: one DVE pass over psum rows 0..VROW pulls both
            # ju_s (row 0: tri-NEG + u*64, aux compensates) and v*64 (row
            # VROW: tri there is zero, ju not yet added). Rows 1..31 are
            # garbage and never read. Everything stays in the x64 domain;
            # the Exp descales via scale=1/64 and the host divides by 64.
            vju = rows.tile([VROW + 1, NW], BF16, tag="vju", name=f"vju{g}")
            nc.vector.scalar_tensor_tensor(
                out=vju[:], in0=ps_d[g][0 : VROW + 1, :], scalar=1.0,
                op0=ALU.mult, in1=aux33[:, OFFL : OFFL + NW],
                op1=ALU.add,
            )
            ju_sb = vju
            # ship ju/v back early (sync queue: its sequencer is idle here;
            # scalar-queue DMAs would block ACT's Exp/Ln pipeline)
            nc.sync.dma_start(juv[0:1, OFFL : OFFL + NW], vju[0:1, :])
            nc.sync.dma_start(juv[1:2, OFFL : OFFL + NW],
                              vju[VROW : VROW + 1, :])
            # ju broadcast per batch: psum_d[j, b-block] += ju_b[j]
            # (row VROW keeps its unmasked garbage; it is excluded from the
            # on-device reduction and the host adds its true contribution)
            for b in range(GROUPS[g]):
                nc.tensor.matmul(
                    ps_d[g][:, b * L : (b + 1) * L],
                    lhsT=ju_sb[0:1, b * L : (b + 1) * L],
                    rhs=ones_row[:],
                    start=False, stop=True,
                    skip_group_check=True,
                )
            nc.scalar.activation(exg[:, OFFL : OFFL + NW], ps_d[g][:],
                                 ACTF.Exp, scale=1.0 / WSCALE)
            junk = exg_pool.tile([L, NW], BF16, tag="junk", name=f"junk{g}")
            nc.scalar.activation(junk[:], exg[:, OFFL : OFFL + NW],
                                 ACTF.Ln, bias=1.0,
                                 accum_out=O[:, g : g + 1])

        # reduce O over partitions on PE -> [1, G] so the final DMA is one
        # fat descriptor instead of 128 x 8B (receipt latency gates teardown)
        ps_rs = ps_w_pool.tile([1, G], F32, tag="psrs")
        nc.tensor.matmul(ps_rs[:], lhsT=ones_col[:], rhs=O[:],
                         start=True, stop=True)
        rs_sb = rows.tile([1, G], F32, tag="rssb")
        nc.vector.tensor_copy(rs_sb[:], ps_rs[:])
        # NOTE: issuing this DMA after the tile teardown overlaps its
        # ~1.5us write-receipt with the sem-file wipe, but the completion
        # increment then races the wipe zeroing its own semaphore and can
        # leave the device wedged (observed NRT_EXEC_UNIT_UNRECOVERABLE on
        # a later execution). Keep it tracked inside the tile context.
        # Scalar queue: ACT is finished by now and the Sync sequencer is
        # still busy issuing the juv DMAs.
        nc.scalar.dma_start(rs[:], rs_sb[:], single_packet=True)

    nc.compile()
    return nc


_NC = None


def _get_nc():
    global _NC
    if _NC is None:
        _NC = build_program()
    return _NC


def _prep(encoder_output, mask, W, b):
    """Host-side prep: shard + pack the DoubleRow fp8 layout."""
    W = np.asarray(W, dtype=np.float32)
    b = np.asarray(b, dtype=np.float32).reshape(2)
    mask = np.asarray(mask)
    c = float(b[1] - b[0])
    lens = mask.astype(np.int64).sum(axis=1)  # [BSZ]
    j = np.arange(L)

    wu = (W[:D, 1] - W[:D, 0]) * WSCALE
    wv = (W[D:, 1] - W[D:, 0]) * WSCALE
    # w2[p, c, t, m]: col 0 = wu, cols 1.. = wv; d = c*256 + t*128 + p
    w2 = np.empty((128, NC_CHUNK, 2, 128), dtype=np.float32)
    w2[:] = wv.reshape(NC_CHUNK, 2, 128).transpose(2, 0, 1)[:, :, :, None]
    w2[:, :, :, 0] = wu.reshape(NC_CHUNK, 2, 128).transpose(2, 0, 1)
    w2 = w2.reshape(128, NC_CHUNK * 2 * 128).astype(ml_dtypes.float8_e4m3)

    ident_b = np.eye(L, dtype=ml_dtypes.bfloat16)
    trineg_f = np.where(j[None, :] < j[:, None], 0.0, NEG).astype(np.float32)
    trineg_f[VROW] = 0.0
    trineg_b = np.tile(trineg_f.astype(ml_dtypes.bfloat16), (1, max(GROUPS)))
    cst = np.concatenate([ident_b.view(np.uint16), trineg_b.view(np.uint16)], axis=1)

    enc_f = np.asarray(encoder_output, dtype=np.float32)
    maps = []
    for cid in range(N_CORES):
        sl = slice(cid * NB, (cid + 1) * NB)
        ec = enc_f[sl]  # [8, 128, 2048]
        # per group: [b, j, c, t, p] -> [p, c, t, b, j], concatenated g-major
        parts = []
        for g in range(len(GROUPS)):
            eg = ec[GOFF[g] : GOFF[g + 1]]  # [Bg, 128, 2048]
            pg = eg.reshape(GROUPS[g], L, NC_CHUNK, 2, 128).transpose(4, 2, 3, 0, 1)
            parts.append(pg.reshape(128, NC_CHUNK * 2 * GROUPS[g] * L))
        ep = np.ascontiguousarray(np.concatenate(parts, axis=1)).astype(
            ml_dtypes.float8_e4m3
        )
        lc = lens[sl]  # [NB]
        # aux_ju = 64*(len-mask + c) - NEGB (compensates tri row 0 in psum)
        auxv = (
            (np.where(j[None, :] < lc[:, None], 0.0, NEG) + c) * WSCALE - NEGB
        ).astype(np.float32).reshape(1, NB * L)
        maps.append(
            {
                "enc": ep,
                "w2": w2,
                "cst": np.ascontiguousarray(cst),
                "auxju": np.ascontiguousarray(auxv),
            }
        )
    return maps, lens


def kernel(encoder_output, mask, W, b, _run_kwargs=None):
    from concourse.bass_utils import run_bass_kernel_spmd

    nc = _get_nc()
    maps, lens = _prep(np.asarray(encoder_output), mask, W, b)
    res = run_bass_kernel_spmd(nc, maps, core_ids=list(range(N_CORES)),
                               **(_run_kwargs or {}))
    pair_sum = np.float64(0.0)
    diag = np.float64(0.0)
    for cid, r in enumerate(res.results):
        pair_sum += np.asarray(r["rs"], dtype=np.float64).sum()
        juv = np.asarray(r["juv"], dtype=np.float64)  # [2, NB*L]
        for bi in range(NB):
            ln = int(lens[cid * NB + bi])
            ju = juv[0, bi * L : bi * L + L] / WSCALE
            v = juv[1, bi * L : bi * L + L] / WSCALE
            diag += ju[1:ln].sum() + v[: ln - 1].sum()
            # row VROW is excluded from the device reduction (left unmasked);
            # add its true pair contribution here
            if ln > VROW:
                pair_sum += np.log1p(np.exp(ju[VROW] + v[:VROW])).sum()
    n_valid = int((lens * (lens - 1) // 2).sum())
    loss = (pair_sum - diag) / max(n_valid, 1)
    out = np.array(loss, dtype=np.float32)
    if _run_kwargs is not None:
        return out, res
    return out


if __name__ == "__main__":
    import reference

    inputs = {k: np.asarray(v) for k, v in reference.setup_inputs().items()}
    print(kernel(**inputs))
